# revision 25
# baseline (speedup 1.0000x reference)
"""Trainium2 Bass kernel for nn_Attention (B=4, SEQ=2048, DIM=1024, H=16).

Sharding v2: (batch x head-half) - core i handles batch i%4, heads
(i//4)*8 .. +8 (four head-pairs). Host sums the 2 partial FC outputs
per batch (+ b_fc). Versus the v1 all-batches-per-core tensor-parallel
split this keeps PE work identical but cuts per-core DMA ~4x (each
core loads only its batch) and removes the HWDGE/SP-queue contention.

Kernel structure per core:
- Host-side key compaction (padding-masked keys contribute exactly 0).
- QKV projections in fp8 hi/lo x DoubleRow (3 passes, 0.5 cyc/col).
- Scores (K=64) and AV (K=128) in bf16; exp on ACT engine.
- Softmax normalization: the AV PSUM rowsum row (augmented-V trick) is
  partition-broadcast on the idle GPSIMD engine directly from PSUM,
  reciprocal'd on DVE, and multiplied into xn - no PE broadcast matmul,
  no staging copies, no partition-shift DMA for the rowsum.
- FC: bf16, 4 accumulating K=128 chunk matmuls per output tile, chunk
  order chosen so the head-b partition-shift DMAs hide under the spine.
- Score exps are paced into AV/Qproj/FC gaps (~1.2us apart) so the ACT
  engine never back-pressures the in-order PE queue via the 2-deep
  score-PSUM pool.
"""

import sys

sys.path.insert(0, "/opt/trn_rl_repo")

from contextlib import ExitStack

import numpy as np
import ml_dtypes

import concourse.bass as bass
import concourse.tile as tile
from concourse import bacc, mybir
from concourse.bass_utils import run_bass_kernel_spmd

F32 = mybir.dt.float32
BF16 = mybir.dt.bfloat16
BF16_NP = ml_dtypes.bfloat16
FP8 = mybir.dt.float8e4
FP8_NP = ml_dtypes.float8_e4m3
DR = mybir.MatmulPerfMode.DoubleRow

B, SEQ, DIM, H, DH = 4, 2048, 1024, 16, 64
SCALE = DH ** -0.5  # 0.125
NHP = 4  # head-pairs per core (8 heads)

_CACHE = {}
LAST_RESULTS = None


def _build(kj):
    """kj: number of 128-wide key tiles after compaction (uniform, padded)."""
    KP = kj * 128

    nc = bacc.Bacc(
        "TRN2",
        target_bir_lowering=False,
        debug=False,
        enable_asserts=False,
        num_devices=8,
    )
    xTh = nc.dram_tensor("xTh", [DIM, SEQ], FP8, kind="ExternalInput").ap()
    xTl = nc.dram_tensor("xTl", [DIM, SEQ], FP8, kind="ExternalInput").ap()
    xkTh = nc.dram_tensor("xkTh", [DIM, KP], FP8, kind="ExternalInput").ap()
    xkTl = nc.dram_tensor("xkTl", [DIM, KP], FP8, kind="ExternalInput").ap()
    wqkvTh = nc.dram_tensor("wqkvTh", [DIM, 384 * NHP], FP8, kind="ExternalInput").ap()
    wqkvTl = nc.dram_tensor("wqkvTl", [DIM, 384 * NHP], FP8, kind="ExternalInput").ap()
    wfcT = nc.dram_tensor("wfcT", [128, NHP * DIM], BF16, kind="ExternalInput").ap()
    keep = nc.dram_tensor("keep", [128, kj], F32, kind="ExternalInput").ap()
    outp = nc.dram_tensor("outp", [SEQ, DIM], BF16, kind="ExternalOutput").ap()

    EXP = mybir.ActivationFunctionType.Exp

    with tile.TileContext(nc) as tc, ExitStack() as ctx:
        p_const = ctx.enter_context(tc.tile_pool(name="const", bufs=1))
        p_xq = ctx.enter_context(tc.tile_pool(name="xq", bufs=2))
        p_xk = ctx.enter_context(tc.tile_pool(name="xk", bufs=1))
        p_k = ctx.enter_context(tc.tile_pool(name="k", bufs=1))
        p_va = ctx.enter_context(tc.tile_pool(name="va", bufs=1))
        p_q = ctx.enter_context(tc.tile_pool(name="q", bufs=3))
        p_pt = ctx.enter_context(tc.tile_pool(name="pt", bufs=26))
        p_xn = ctx.enter_context(tc.tile_pool(name="xn", bufs=8))
        p_xnb = ctx.enter_context(tc.tile_pool(name="xnb", bufs=4))
        p_rb = ctx.enter_context(tc.tile_pool(name="rb", bufs=3))
        p_fo = ctx.enter_context(tc.tile_pool(name="fo", bufs=3))
        p_st = ctx.enter_context(tc.tile_pool(name="st", bufs=2, space="PSUM"))
        p_xa = ctx.enter_context(tc.tile_pool(name="xa", bufs=2, space="PSUM"))
        p_mm = ctx.enter_context(tc.tile_pool(name="mm", bufs=2, space="PSUM"))

        # ---- constant + input loads (SP queue) ----
        # The K-side tensors load in c-chunk halves, interleaved so the
        # pass-major startup K projection can start matmuls on (whA, xkhA)
        # ~4us in instead of waiting for all four full tensors.
        WQC = 384 * NHP
        wh_sb = p_const.tile([128, 8 * WQC], FP8, tag="wh")
        wl_sb = p_const.tile([128, 8 * WQC], FP8, tag="wl")
        xkh_sb = p_xk.tile([128, 8 * KP], FP8, tag="xkh")
        xkl_sb = p_xk.tile([128, 8 * KP], FP8, tag="xkl")

        def half_load(sb, src, nwide, h):
            cs = slice(h * 4 * 128, (h + 1) * 4 * 128)
            nc.sync.dma_start(
                sb[:, h * 4 * nwide : (h + 1) * 4 * nwide].rearrange(
                    "p (c n) -> p c n", c=4
                ),
                src[cs, :].rearrange("(c p) n -> p c n", c=4),
            )

        keep_sb = p_const.tile([128, kj], F32, tag="keep")
        nc.sync.dma_start(keep_sb[:], keep[:])
        half_load(wh_sb, wqkvTh, WQC, 0)
        half_load(xkh_sb, xkTh, KP, 0)
        half_load(wh_sb, wqkvTh, WQC, 1)
        half_load(xkh_sb, xkTh, KP, 1)
        half_load(xkl_sb, xkTl, KP, 0)
        half_load(xkl_sb, xkTl, KP, 1)
        half_load(wl_sb, wqkvTl, WQC, 0)
        half_load(wl_sb, wqkvTl, WQC, 1)
        w3h = wh_sb[:].rearrange("p (c n) -> p c n", c=8)
        w3l = wl_sb[:].rearrange("p (c n) -> p c n", c=8)
        xk3h = xkh_sb[:].rearrange("p (c n) -> p c n", c=8)
        xk3l = xkl_sb[:].rearrange("p (c n) -> p c n", c=8)

        def load_xq(qt):
            t = {}
            cs = slice(qt * 1024, (qt + 1) * 1024)
            for tagv, src in (("xqh", xTh), ("xql", xTl)):
                xt = p_xq.tile([128, 8 * 1024], FP8, tag=tagv)
                nc.sync.dma_start(
                    xt[:].rearrange("p (c n) -> p c n", c=8),
                    src[:, cs].rearrange("(c p) n -> p c n", c=8),
                )
                t[tagv[-1]] = xt
            return t

        xq_t = {0: load_xq(0), 1: load_xq(1)}
        wfc_sb = p_const.tile([128, NHP * DIM], BF16, tag="wfc")
        nc.sync.dma_start(wfc_sb[:], wfcT[:])

        # ---- K projection: kT2[hp] = [128 (2 heads x 64 dh), KP] bf16 ----
        def kproj(hp):
            kT2 = p_k.tile([128, KP], BF16, tag=f"k{hp}")
            wo = hp * 384 + 128
            n0 = 0
            while n0 < KP:
                n1 = min(n0 + 512, KP)
                ps = p_mm.tile([128, 512], F32, tag="mm")
                first = True
                for wv, xv in ((w3h, xk3h), (w3h, xk3l), (w3l, xk3h)):
                    for cp in range(0, 8, 2):
                        nc.tensor.matmul(
                            ps[:, : n1 - n0],
                            wv[:, cp : cp + 2, wo : wo + 128],
                            xv[:, cp : cp + 2, n0:n1],
                            start=first,
                            stop=(wv is w3l and cp == 6),
                            perf_mode=DR,
                        )
                        first = False
                nc.scalar.copy(kT2[:, n0:n1], ps[:, : n1 - n0])
                n0 = n1
            return kT2

        # ---- V projection into keep-scaled augmented layout ----
        # va[hp] columns per key tile t: [v_a(64)*keep, keep, v_b(64)*keep, keep]
        def vproj(hp, pump):
            va = p_va.tile([128, kj * 130], BF16, tag=f"va{hp}")
            wo = hp * 384 + 256
            for t in range(kj):
                pv = p_mm.tile([128, 128], F32, tag="mm")
                first = True
                for xv, wv in ((xk3h, w3h), (xk3l, w3h), (xk3h, w3l)):
                    for cp in range(0, 8, 2):
                        nc.tensor.matmul(
                            pv[:],
                            xv[:, cp : cp + 2, t * 128 : (t + 1) * 128],
                            wv[:, cp : cp + 2, wo : wo + 128],
                            start=first,
                            stop=(wv is w3l and cp == 6),
                            perf_mode=DR,
                        )
                        first = False
                kap = keep_sb[:, t : t + 1]
                o = t * 130
                nc.vector.tensor_scalar_mul(va[:, o : o + 64], pv[:, 0:64], kap)
                nc.vector.tensor_copy(va[:, o + 64 : o + 65], kap)
                nc.vector.tensor_scalar_mul(va[:, o + 65 : o + 129], pv[:, 64:128], kap)
                nc.vector.tensor_copy(va[:, o + 129 : o + 130], kap)
                if t % 2 == 1:
                    pump(1)
            return va

        # ---- Q projection: qS = [128 (2 heads x 64 dh), 1024 queries] ----
        def qproj(qt, hp, pump):
            qS = p_q.tile([128, 1024], BF16, tag="q")
            wo = hp * 384
            xq3h = xq_t[qt]["h"][:].rearrange("p (c n) -> p c n", c=8)
            xq3l = xq_t[qt]["l"][:].rearrange("p (c n) -> p c n", c=8)
            for n in range(2):
                ps = p_mm.tile([128, 512], F32, tag="mm")
                first = True
                for wv, xv in ((w3h, xq3h), (w3h, xq3l), (w3l, xq3h)):
                    for cp in range(0, 8, 2):
                        nc.tensor.matmul(
                            ps[:],
                            wv[:, cp : cp + 2, wo : wo + 128],
                            xv[:, cp : cp + 2, n * 512 : (n + 1) * 512],
                            start=first,
                            stop=(wv is w3l and cp == 6),
                            perf_mode=DR,
                        )
                        first = False
                        if wv is w3h and xv is xq3l and cp == 6:
                            pump(1)
                nc.vector.tensor_copy(qS[:, n * 512 : (n + 1) * 512], ps[:])
                pump(1)
            return qS

        pts = {}
        xns = {}
        kT2s = []
        vas = []
        fc_queue = []

        class SS:
            """Pending score+exp tiles for one (qt, hp), paced into PE gaps."""

            def __init__(self, qt, hp, kT2, qS):
                self.qt, self.hp, self.kT2, self.qS = qt, hp, kT2, qS
                self.jobs = [(a, t) for a in range(2) for t in range(kj)]

            def _emit(self, a, t):
                st = p_st.tile([128, 1024], F32, tag="st")
                for hh in range(2):
                    nc.tensor.matmul(
                        st[:, hh * 512 : (hh + 1) * 512],
                        self.kT2[a * 64 : (a + 1) * 64, t * 128 : (t + 1) * 128],
                        self.qS[a * 64 : (a + 1) * 64, hh * 512 : (hh + 1) * 512],
                        start=True,
                        stop=True,
                        skip_group_check=True,
                    )
                pt = p_pt.tile([128, 1024], BF16, tag="pt")
                nc.scalar.activation(pt[:], st[:], EXP, scale=SCALE / 256.0)
                pts[(self.qt, self.hp, a, t)] = pt

            def pump(self, n=1):
                while n > 0 and self.jobs:
                    self._emit(*self.jobs.pop(0))
                    n -= 1

            def flush_all(self):
                while self.jobs:
                    self._emit(*self.jobs.pop(0))

        def emit_fc(ss=None):
            if not fc_queue:
                return
            qt, qq = fc_queue.pop(0)
            last = qt == 1 and not fc_queue
            order = (0, 1, 2, 3) if qt == 0 else (3, 0, 1, 2)
            fo = p_fo.tile([128, DIM], BF16, tag="fo")
            r0 = qt * 1024 + qq * 128
            for ot in range(2):
                fp = p_mm.tile([128, 512], F32, tag="mm")
                for j, hp in enumerate(order):
                    nc.tensor.matmul(
                        fp[:],
                        xns[(qt, hp)][:, qq * 128 : (qq + 1) * 128],
                        wfc_sb[:, hp * DIM + ot * 512 : hp * DIM + ot * 512 + 512],
                        start=(j == 0),
                        stop=(j == 3),
                    )
                nc.vector.tensor_copy(fo[:, ot * 512 : (ot + 1) * 512], fp[:])
                if last:
                    # final tile: store each half as soon as it is staged
                    nc.gpsimd.dma_start(
                        outp[r0 : r0 + 128, ot * 512 : (ot + 1) * 512],
                        fo[:, ot * 512 : (ot + 1) * 512],
                    )
                if ss is not None:
                    ss.pump(1)
            if not last:
                nc.gpsimd.dma_start(outp[r0 : r0 + 128, :], fo[:])

        def av_block(qt, hp, ss_next):
            xn = p_xn.tile([128, 1024], BF16, tag="xn", name=f"xn{qt}{hp}")
            xns[(qt, hp)] = xn
            va = vas[hp]
            for a in range(2):
                for qh in range(2):
                    xa = p_xa.tile([65, 512], F32, tag="xa")
                    for t in range(kj):
                        o = t * 130 + a * 65
                        nc.tensor.matmul(
                            xa[:],
                            va[:, o : o + 65],
                            pts[(qt, hp, a, t)][:, qh * 512 : (qh + 1) * 512],
                            start=(t == 0),
                            stop=(t == kj - 1),
                            skip_group_check=True,
                        )
                        if t % 4 == 3 and ss_next is not None and len(ss_next.jobs) > 10:
                            ss_next.pump(1)
                    # normalize: stage the PSUM rowsum row to SBUF, take the
                    # reciprocal on DVE, partition-broadcast on GPSIMD,
                    # multiply into xn.
                    rs = p_rb.tile([1, 512], F32, tag="rs")
                    nc.vector.tensor_copy(rs[0:1, :], xa[64:65, :])
                    rr = p_rb.tile([1, 512], F32, tag="rr")
                    nc.vector.reciprocal_approx_fast(rr[0:1, :], rs[0:1, :])
                    ri = p_rb.tile([64, 512], F32, tag="ri")
                    nc.gpsimd.partition_broadcast(ri[:], rr[0:1, :])
                    if a == 0:
                        nc.vector.tensor_mul(
                            xn[0:64, qh * 512 : (qh + 1) * 512], xa[0:64, :], ri[:]
                        )
                    else:
                        xnb = p_xnb.tile([64, 512], BF16, tag="xnb")
                        nc.vector.tensor_mul(xnb[:], xa[0:64, :], ri[:])
                        # shift DMA on SP (stores go via GPSIMD SWDGE so they
                        # cannot delay these latency-critical shifts).
                        nc.sync.dma_start(
                            xn[64:128, qh * 512 : (qh + 1) * 512], xnb[:]
                        )
                        emit_fc(ss_next)
                        emit_fc(ss_next)
                    if ss_next is not None:
                        ss_next.pump(3)

        # hp0+hp1 K projections pass-major: the hh-pass matmuls for all
        # chunks of both head-pairs run as soon as (wh, xkh) land, the
        # hl-pass when xkl lands, the lh-pass when wl lands - instead of
        # the whole pipeline stalling on the last of the four loads.
        def kproj01():
            res, ps2, pc2 = [], {}, {}
            nch = [
                (i * 512, min((i + 1) * 512, KP))
                for i in range((KP + 511) // 512)
            ]
            for hp in (0, 1):
                res.append(
                    p_k.tile([128, KP], BF16, tag=f"k{hp}", name=f"kT2_{hp}")
                )
                ps2[hp] = p_st.tile([128, 1024], F32, tag="st", name=f"kps{hp}")
                if KP > 1024:
                    pc2[hp] = p_mm.tile([128, 512], F32, tag="mm", name=f"kpc{hp}")
            for pi, (wv, xv) in enumerate(
                ((w3h, xk3h), (w3h, xk3l), (w3l, xk3h))
            ):
                for cph in (0, 4):
                    for hp in (0, 1):
                        wo = hp * 384 + 128
                        for ci, (n0, n1) in enumerate(nch):
                            out = (
                                ps2[hp][:, n0:n1]
                                if ci < 2
                                else pc2[hp][:, : n1 - n0]
                            )
                            for cp in (cph, cph + 2):
                                nc.tensor.matmul(
                                    out,
                                    wv[:, cp : cp + 2, wo : wo + 128],
                                    xv[:, cp : cp + 2, n0:n1],
                                    start=(pi == 0 and cp == 0),
                                    stop=(pi == 2 and cp == 6),
                                    perf_mode=DR,
                                    skip_group_check=True,
                                )
            for hp in (0, 1):
                e = min(KP, 1024)
                # ACT is idle during startup - keep DVE free for qS copies
                nc.scalar.copy(res[hp][:, 0:e], ps2[hp][:, 0:e])
                if KP > 1024:
                    nc.scalar.copy(res[hp][:, 1024:KP], pc2[hp][:, : KP - 1024])
            return res

        # ================= spine =================
        if KP <= 1536:
            kT2s.extend(kproj01())
        else:
            kT2s.append(kproj(0))
            kT2s.append(kproj(1))
        kT2s.append(kproj(2))
        kT2s.append(kproj(3))

        qS0 = qproj(0, 0, lambda n: None)
        S = {(0, 0): SS(0, 0, kT2s[0], qS0)}
        qSs = {(0, 0): qS0}

        for hp in range(NHP):
            vas.append(vproj(hp, S[(0, 0)].pump))
        S[(0, 0)].flush_all()

        prev = (0, 0)
        seq = [(0, 1), (0, 2), (0, 3), (1, 3), (1, 0), (1, 1), (1, 2)]
        for qt, hp in seq:
            qS = qproj(qt, hp, S[prev].pump)
            qSs[(qt, hp)] = qS
            cur = SS(qt, hp, kT2s[hp], qS)
            S[(qt, hp)] = cur
            S[prev].flush_all()
            av_block(prev[0], prev[1], cur)
            if prev == (0, 3):
                fc_queue.extend((0, qq) for qq in range(8))
            while len(fc_queue) > 4:
                emit_fc(cur)
            prev = (qt, hp)

        S[prev].flush_all()
        fc_queue.extend((1, qq) for qq in range(8))
        av_block(prev[0], prev[1], None)
        while fc_queue:
            emit_fc()

    nc.compile()
    return nc


def _hilo(a):
    hi = a.astype(FP8_NP)
    lo = (a - hi.astype(np.float32)).astype(FP8_NP)
    return hi, lo


def _prep_inputs(inputs, W_qkv, W_fc, padding_mask, kj):
    KP = kj * 128
    x = np.asarray(inputs, np.float32)
    Wq = np.asarray(W_qkv, np.float32)
    Wf = np.asarray(W_fc, np.float32)
    mask = np.asarray(padding_mask)

    xT, xkT, keepc = {}, {}, {}
    for b in range(B):
        xb = x[b]
        xT[b] = _hilo(np.ascontiguousarray(xb.T))
        idx = np.nonzero(mask[b] == 0)[0]
        rows = np.zeros((KP, DIM), np.float32)
        rows[: len(idx)] = xb[idx]
        xkT[b] = _hilo(np.ascontiguousarray(rows.T))
        kv = np.zeros(KP, np.float32)
        kv[: len(idx)] = 1.0
        keepc[b] = np.ascontiguousarray(kv.reshape(kj, 128).T)

    in_maps = []
    for i in range(8):
        b, hs = i % 4, i // 4
        qrs = Wq[hs * 512 : (hs + 1) * 512]
        krs = Wq[DIM + hs * 512 : DIM + (hs + 1) * 512]
        vrs = Wq[2 * DIM + hs * 512 : 2 * DIM + (hs + 1) * 512]
        blocks = []
        for hp in range(NHP):
            blocks += [
                qrs[hp * 128 : (hp + 1) * 128],
                krs[hp * 128 : (hp + 1) * 128],
                vrs[hp * 128 : (hp + 1) * 128],
            ]
        wT = np.ascontiguousarray(np.concatenate(blocks, axis=0).T) * 16.0
        wh, wl = _hilo(wT)
        wfcT = np.concatenate(
            [
                np.ascontiguousarray(
                    Wf[:, hs * 512 + hp * 128 : hs * 512 + (hp + 1) * 128].T
                )
                for hp in range(NHP)
            ],
            axis=1,
        ) / 16.0
        in_maps.append(
            {
                "xTh": xT[b][0],
                "xTl": xT[b][1],
                "xkTh": xkT[b][0],
                "xkTl": xkT[b][1],
                "wqkvTh": wh,
                "wqkvTl": wl,
                "wfcT": wfcT.astype(BF16_NP),
                "keep": keepc[b],
            }
        )
    return in_maps


def kernel(inputs, W_qkv, W_fc, b_fc, padding_mask, trace=False, trace_kwargs=None):
    global LAST_RESULTS
    mask = np.asarray(padding_mask)
    kj = max(
        1, max(int(np.ceil((mask[b] == 0).sum() / 128)) for b in range(B))
    )
    if kj not in _CACHE:
        _CACHE[kj] = _build(kj)
    nc = _CACHE[kj]
    _CACHE["nc"] = nc  # last-used, for external profiling
    in_maps = _prep_inputs(inputs, W_qkv, W_fc, padding_mask, kj)
    kw = {}
    if trace:
        kw["trace"] = True
        if trace_kwargs:
            kw.update(trace_kwargs)
    res = run_bass_kernel_spmd(nc, in_maps, core_ids=list(range(8)), **kw)
    LAST_RESULTS = res
    out = np.empty((B, SEQ, DIM), np.float32)
    bfc = np.asarray(b_fc, np.float32)[None, :]
    for b in range(B):
        out[b] = (
            res.results[b]["outp"].astype(np.float32)
            + res.results[b + 4]["outp"].astype(np.float32)
            + bfc
        )
    return out


# revision 41
# speedup vs baseline: 1.0274x; 1.0274x over previous
"""Trainium2 Bass kernel for nn_Attention (B=4, SEQ=2048, DIM=1024, H=16).

Sharding v2: (batch x head-half) - core i handles batch i%4, heads
(i//4)*8 .. +8 (four head-pairs). Host sums the 2 partial FC outputs
per batch (+ b_fc). Versus the v1 all-batches-per-core tensor-parallel
split this keeps PE work identical but cuts per-core DMA ~4x (each
core loads only its batch) and removes the HWDGE/SP-queue contention.

Kernel structure per core:
- Host-side key compaction (padding-masked keys contribute exactly 0).
- QKV projections in fp8 hi/lo x DoubleRow (3 passes, 0.5 cyc/col).
- Scores (K=64) and AV (K=128) in bf16; exp on ACT engine.
- Softmax normalization: the AV PSUM rowsum row (augmented-V trick) is
  partition-broadcast on the idle GPSIMD engine directly from PSUM,
  reciprocal'd on DVE, and multiplied into xn - no PE broadcast matmul,
  no staging copies, no partition-shift DMA for the rowsum.
- FC: bf16, 4 accumulating K=128 chunk matmuls per output tile, chunk
  order chosen so the head-b partition-shift DMAs hide under the spine.
- Score exps are paced into AV/Qproj/FC gaps (~1.2us apart) so the ACT
  engine never back-pressures the in-order PE queue via the 2-deep
  score-PSUM pool.
"""

import sys

sys.path.insert(0, "/opt/trn_rl_repo")

from contextlib import ExitStack

import numpy as np
import ml_dtypes

import concourse.bass as bass
import concourse.tile as tile
from concourse import bacc, mybir
from concourse.bass_utils import run_bass_kernel_spmd

F32 = mybir.dt.float32
BF16 = mybir.dt.bfloat16
BF16_NP = ml_dtypes.bfloat16
FP8 = mybir.dt.float8e4
FP8_NP = ml_dtypes.float8_e4m3
DR = mybir.MatmulPerfMode.DoubleRow

B, SEQ, DIM, H, DH = 4, 2048, 1024, 16, 64
SCALE = DH ** -0.5  # 0.125
NHP = 4  # head-pairs per core (8 heads)

_CACHE = {}
LAST_RESULTS = None


def _build(kj):
    """kj: number of 128-wide key tiles after compaction (uniform, padded)."""
    KP = kj * 128

    nc = bacc.Bacc(
        "TRN2",
        target_bir_lowering=False,
        debug=False,
        enable_asserts=False,
        num_devices=8,
    )
    xTh = nc.dram_tensor("xTh", [DIM, SEQ], FP8, kind="ExternalInput").ap()
    xTl = nc.dram_tensor("xTl", [DIM, SEQ], FP8, kind="ExternalInput").ap()
    xkTh = nc.dram_tensor("xkTh", [DIM, KP], FP8, kind="ExternalInput").ap()
    xkTl = nc.dram_tensor("xkTl", [DIM, KP], FP8, kind="ExternalInput").ap()
    wqkvTh = nc.dram_tensor("wqkvTh", [DIM, 384 * NHP], FP8, kind="ExternalInput").ap()
    wqkvTl = nc.dram_tensor("wqkvTl", [DIM, 384 * NHP], FP8, kind="ExternalInput").ap()
    wfcT = nc.dram_tensor("wfcT", [128, NHP * DIM], BF16, kind="ExternalInput").ap()
    keep = nc.dram_tensor("keep", [128, kj], F32, kind="ExternalInput").ap()
    outp = nc.dram_tensor("outp", [SEQ, DIM], BF16, kind="ExternalOutput").ap()

    EXP = mybir.ActivationFunctionType.Exp

    with tile.TileContext(nc) as tc, ExitStack() as ctx:
        p_const = ctx.enter_context(tc.tile_pool(name="const", bufs=1))
        p_xq = ctx.enter_context(tc.tile_pool(name="xq", bufs=2))
        p_xk = ctx.enter_context(tc.tile_pool(name="xk", bufs=1))
        p_k = ctx.enter_context(tc.tile_pool(name="k", bufs=1))
        p_va = ctx.enter_context(tc.tile_pool(name="va", bufs=1))
        p_q = ctx.enter_context(tc.tile_pool(name="q", bufs=3))
        p_pt = ctx.enter_context(tc.tile_pool(name="pt", bufs=26))
        p_xn = ctx.enter_context(tc.tile_pool(name="xn", bufs=8))
        p_xnb = ctx.enter_context(tc.tile_pool(name="xnb", bufs=4))
        p_rb = ctx.enter_context(tc.tile_pool(name="rb", bufs=3))
        p_fo = ctx.enter_context(tc.tile_pool(name="fo", bufs=3))
        p_st = ctx.enter_context(tc.tile_pool(name="st", bufs=2, space="PSUM"))
        p_xa = ctx.enter_context(tc.tile_pool(name="xa", bufs=2, space="PSUM"))
        p_mm = ctx.enter_context(tc.tile_pool(name="mm", bufs=2, space="PSUM"))

        # ---- constant + input loads (SP queue) ----
        # The K-side tensors load in c-chunk halves, interleaved so the
        # pass-major startup K projection can start matmuls on (whA, xkhA)
        # ~4us in instead of waiting for all four full tensors.
        WQC = 384 * NHP
        wh_sb = p_const.tile([128, 8 * WQC], FP8, tag="wh")
        wl_sb = p_const.tile([128, 8 * WQC], FP8, tag="wl")
        xkh_sb = p_xk.tile([128, 8 * KP], FP8, tag="xkh")
        xkl_sb = p_xk.tile([128, 8 * KP], FP8, tag="xkl")

        def wcol_load(sb, src, a, b):
            nc.sync.dma_start(
                sb[:].rearrange("p (c n) -> p c n", c=8)[:, :, a:b],
                src[:, a:b].rearrange("(c p) n -> p c n", c=8),
            )

        def xk_load(sb, src):
            nc.sync.dma_start(
                sb[:].rearrange("p (c n) -> p c n", c=8),
                src[:].rearrange("(c p) n -> p c n", c=8),
            )

        wcol_load(wh_sb, wqkvTh, 0, 512)  # K weights (startup-critical)
        xk_load(xkh_sb, xkTh)
        xk_load(xkl_sb, xkTl)
        wcol_load(wl_sb, wqkvTl, 0, 512)
        wcol_load(wh_sb, wqkvTh, 512, 1024)  # Q weights
        wcol_load(wl_sb, wqkvTl, 512, 1024)
        keep_sb = p_const.tile([128, kj], F32, tag="keep")
        nc.sync.dma_start(keep_sb[:], keep[:])
        w3h = wh_sb[:].rearrange("p (c n) -> p c n", c=8)
        w3l = wl_sb[:].rearrange("p (c n) -> p c n", c=8)
        xk3h = xkh_sb[:].rearrange("p (c n) -> p c n", c=8)
        xk3l = xkl_sb[:].rearrange("p (c n) -> p c n", c=8)

        def load_xq(qt):
            t = {}
            cs = slice(qt * 1024, (qt + 1) * 1024)
            for tagv, src in (("xqh", xTh), ("xql", xTl)):
                xt = p_xq.tile([128, 8 * 1024], FP8, tag=tagv)
                nc.sync.dma_start(
                    xt[:].rearrange("p (c n) -> p c n", c=8),
                    src[:, cs].rearrange("(c p) n -> p c n", c=8),
                )
                t[tagv[-1]] = xt
            return t

        xq_t = {0: load_xq(0)}
        wcol_load(wh_sb, wqkvTh, 1024, 1536)  # V weights (needed latest)
        wcol_load(wl_sb, wqkvTl, 1024, 1536)
        xq_t[1] = load_xq(1)
        wfc_sb = p_const.tile([128, NHP * DIM], BF16, tag="wfc")
        nc.sync.dma_start(wfc_sb[:], wfcT[:])

        # ---- K projection: kT2[hp] = [128 (2 heads x 64 dh), KP] bf16 ----
        def kproj(hp):
            kT2 = p_k.tile([128, KP], BF16, tag=f"k{hp}")
            wo = hp * 128
            n0 = 0
            while n0 < KP:
                n1 = min(n0 + 512, KP)
                ps = p_mm.tile([128, 512], F32, tag="mm")
                first = True
                for wv, xv in ((w3h, xk3h), (w3h, xk3l), (w3l, xk3h)):
                    for cp in range(0, 8, 2):
                        nc.tensor.matmul(
                            ps[:, : n1 - n0],
                            wv[:, cp : cp + 2, wo : wo + 128],
                            xv[:, cp : cp + 2, n0:n1],
                            start=first,
                            stop=(wv is w3l and cp == 6),
                            perf_mode=DR,
                        )
                        first = False
                nc.scalar.copy(kT2[:, n0:n1], ps[:, : n1 - n0])
                n0 = n1
            return kT2

        # ---- V projection into keep-scaled augmented layout ----
        # va[hp] columns per key tile t: [v_a(64)*keep, keep, v_b(64)*keep, keep]
        def vproj(hp, pump):
            va = p_va.tile([128, kj * 130], BF16, tag=f"va{hp}")
            wo = 1024 + hp * 128
            for t in range(kj):
                pv = p_mm.tile([128, 128], F32, tag="mm")
                first = True
                for xv, wv in ((xk3h, w3h), (xk3l, w3h), (xk3h, w3l)):
                    for cp in range(0, 8, 2):
                        nc.tensor.matmul(
                            pv[:],
                            xv[:, cp : cp + 2, t * 128 : (t + 1) * 128],
                            wv[:, cp : cp + 2, wo : wo + 128],
                            start=first,
                            stop=(wv is w3l and cp == 6),
                            perf_mode=DR,
                        )
                        first = False
                kap = keep_sb[:, t : t + 1]
                o = t * 130
                nc.vector.tensor_scalar_mul(va[:, o : o + 64], pv[:, 0:64], kap)
                nc.vector.tensor_copy(va[:, o + 64 : o + 65], kap)
                nc.vector.tensor_scalar_mul(va[:, o + 65 : o + 129], pv[:, 64:128], kap)
                nc.vector.tensor_copy(va[:, o + 129 : o + 130], kap)
                if t % 2 == 1:
                    pump(1)
            return va

        # ---- Q projection: qS = [128 (2 heads x 64 dh), 1024 queries] ----
        def qproj(qt, hp, pump):
            qS = p_q.tile([128, 1024], BF16, tag="q")
            wo = 512 + hp * 128
            xq3h = xq_t[qt]["h"][:].rearrange("p (c n) -> p c n", c=8)
            xq3l = xq_t[qt]["l"][:].rearrange("p (c n) -> p c n", c=8)
            for n in range(2):
                ps = p_mm.tile([128, 512], F32, tag="mm")
                # xq-lo is the last load to arrive at startup, so the pass
                # that consumes it goes last.
                for pi, (wv, xv) in enumerate(
                    ((w3h, xq3h), (w3l, xq3h), (w3h, xq3l))
                ):
                    for cp in range(0, 8, 2):
                        nc.tensor.matmul(
                            ps[:],
                            wv[:, cp : cp + 2, wo : wo + 128],
                            xv[:, cp : cp + 2, n * 512 : (n + 1) * 512],
                            start=(pi == 0 and cp == 0),
                            stop=(pi == 2 and cp == 6),
                            perf_mode=DR,
                        )
                        if pi == 1 and cp == 6:
                            pump(1)
                nc.vector.tensor_copy(qS[:, n * 512 : (n + 1) * 512], ps[:])
                pump(1)
            return qS

        pts = {}
        xns = {}
        xnbs = {}
        kT2s = []
        vas = []
        fc_queue = []

        pending = []  # streams with unemitted score jobs, FIFO priority

        def pump_any(n=1):
            while n > 0 and pending:
                s = pending[0]
                if not s.jobs:
                    pending.pop(0)
                    continue
                s._emit(*s.jobs.pop(0))
                n -= 1

        def pending_jobs():
            return sum(len(s.jobs) for s in pending)

        class SS:
            """Pending score+exp tiles for one (qt, hp), paced into PE gaps."""

            def __init__(self, qt, hp, kT2, qS):
                self.qt, self.hp, self.kT2, self.qS = qt, hp, kT2, qS
                self.jobs = [(a, t) for a in range(2) for t in range(kj)]
                pending.append(self)

            def _emit(self, a, t):
                st = p_st.tile([128, 1024], F32, tag="st")
                for hh in range(2):
                    nc.tensor.matmul(
                        st[:, hh * 512 : (hh + 1) * 512],
                        self.kT2[a * 64 : (a + 1) * 64, t * 128 : (t + 1) * 128],
                        self.qS[a * 64 : (a + 1) * 64, hh * 512 : (hh + 1) * 512],
                        start=True,
                        stop=True,
                        skip_group_check=True,
                    )
                pt = p_pt.tile([128, 1024], BF16, tag="pt")
                nc.scalar.activation(pt[:], st[:], EXP, scale=SCALE / 256.0)
                pts[(self.qt, self.hp, a, t)] = pt

            def pump(self, n=1):
                while n > 0 and self.jobs:
                    self._emit(*self.jobs.pop(0))
                    n -= 1

            def flush_all(self):
                while self.jobs:
                    self._emit(*self.jobs.pop(0))

        def emit_fc(ss=None):
            if not fc_queue:
                return
            qt, qq = fc_queue.pop(0)
            last = qt == 1 and not fc_queue
            order = (0, 1, 2, 3) if qt == 0 else (3, 0, 1, 2)
            fo = p_fo.tile([128, DIM], BF16, tag="fo")
            r0 = qt * 1024 + qq * 128
            for ot in range(2):
                fp = p_mm.tile([128, 512], F32, tag="mm")
                for j, hp in enumerate(order):
                    nc.tensor.matmul(
                        fp[:],
                        xns[(qt, hp)][:, qq * 128 : (qq + 1) * 128],
                        wfc_sb[:, hp * DIM + ot * 512 : hp * DIM + ot * 512 + 512],
                        start=(j == 0),
                        stop=(j == 3),
                    )
                if last:
                    # final tile: copies on two engines, halves stored as
                    # staged, via the fast HWDGE path (ACT is drained by now)
                    if ot == 0:
                        nc.vector.tensor_copy(fo[:, 0:512], fp[:])
                    else:
                        nc.scalar.copy(fo[:, 512:1024], fp[:])
                    nc.sync.dma_start(
                        outp[r0 : r0 + 128, ot * 512 : (ot + 1) * 512],
                        fo[:, ot * 512 : (ot + 1) * 512],
                    )
                else:
                    nc.vector.tensor_copy(fo[:, ot * 512 : (ot + 1) * 512], fp[:])
                if ss:
                    pump_any(1)
            if not last:
                nc.gpsimd.dma_start(outp[r0 : r0 + 128, :], fo[:])

        def av_block(qt, hp, ss_next):
            xn = p_xn.tile([128, 1024], BF16, tag="xn", name=f"xn{qt}{hp}")
            xns[(qt, hp)] = xn
            va = vas[hp]
            for a in range(2):
                for qh in range(2):
                    xa = p_xa.tile([65, 512], F32, tag="xa")
                    for t in range(kj):
                        o = t * 130 + a * 65
                        nc.tensor.matmul(
                            xa[:],
                            va[:, o : o + 65],
                            pts[(qt, hp, a, t)][:, qh * 512 : (qh + 1) * 512],
                            start=(t == 0),
                            stop=(t == kj - 1),
                            skip_group_check=True,
                        )
                        if t % 4 == 3 and ss_next and pending_jobs() > 10:
                            pump_any(1)
                    # normalize: stage the PSUM rowsum row to SBUF, take the
                    # reciprocal on DVE, partition-broadcast on GPSIMD,
                    # multiply into xn.
                    rs = p_rb.tile([1, 512], F32, tag="rs")
                    nc.vector.tensor_copy(rs[0:1, :], xa[64:65, :])
                    rr = p_rb.tile([1, 512], F32, tag="rr")
                    nc.vector.reciprocal_approx_fast(rr[0:1, :], rs[0:1, :])
                    ri = p_rb.tile([64, 512], F32, tag="ri")
                    nc.gpsimd.partition_broadcast(ri[:], rr[0:1, :])
                    if a == 0:
                        nc.vector.tensor_mul(
                            xn[0:64, qh * 512 : (qh + 1) * 512], xa[0:64, :], ri[:]
                        )
                    else:
                        xnb = p_xnb.tile([64, 512], BF16, tag="xnb")
                        nc.vector.tensor_mul(xnb[:], xa[0:64, :], ri[:])
                        # shift DMA on SP (stores go via GPSIMD SWDGE so
                        # they cannot delay these latency-critical shifts)
                        nc.sync.dma_start(
                            xn[64:128, qh * 512 : (qh + 1) * 512], xnb[:]
                        )
                        emit_fc(ss_next)
                        emit_fc(ss_next)
                    if ss_next:
                        pump_any(3)

        # hp0+hp1 K projections pass-major: the hh-pass matmuls for all
        # chunks of both head-pairs run as soon as (wh, xkh) land, the
        # hl-pass when xkl lands, the lh-pass when wl lands - instead of
        # the whole pipeline stalling on the last of the four loads.
        def kproj01():
            res, ps2, pc2 = [], {}, {}
            nch = [
                (i * 512, min((i + 1) * 512, KP))
                for i in range((KP + 511) // 512)
            ]
            for hp in (0, 1):
                res.append(
                    p_k.tile([128, KP], BF16, tag=f"k{hp}", name=f"kT2_{hp}")
                )
                ps2[hp] = p_st.tile([128, 1024], F32, tag="st", name=f"kps{hp}")
                if KP > 1024:
                    pc2[hp] = p_mm.tile([128, 512], F32, tag="mm", name=f"kpc{hp}")
            for pi, (wv, xv) in enumerate(
                ((w3h, xk3h), (w3h, xk3l), (w3l, xk3h))
            ):
                for cph in (0, 4):
                    for hp in (0, 1):
                        wo = hp * 128
                        for ci, (n0, n1) in enumerate(nch):
                            out = (
                                ps2[hp][:, n0:n1]
                                if ci < 2
                                else pc2[hp][:, : n1 - n0]
                            )
                            for cp in (cph, cph + 2):
                                nc.tensor.matmul(
                                    out,
                                    wv[:, cp : cp + 2, wo : wo + 128],
                                    xv[:, cp : cp + 2, n0:n1],
                                    start=(pi == 0 and cp == 0),
                                    stop=(pi == 2 and cp == 6),
                                    perf_mode=DR,
                                    skip_group_check=True,
                                )
            for hp in (0, 1):
                e = min(KP, 1024)
                # ACT is idle during startup - keep DVE free for qS copies
                nc.scalar.copy(res[hp][:, 0:e], ps2[hp][:, 0:e])
                if KP > 1024:
                    nc.scalar.copy(res[hp][:, 1024:KP], pc2[hp][:, : KP - 1024])
            return res

        # ================= spine =================
        if KP <= 1536:
            kT2s.extend(kproj01())
        else:
            kT2s.append(kproj(0))
            kT2s.append(kproj(1))
        kT2s.append(kproj(2))
        kT2s.append(kproj(3))

        qS0 = qproj(0, 0, lambda n: None)
        S = {(0, 0): SS(0, 0, kT2s[0], qS0)}

        for hp in range(NHP):
            vas.append(vproj(hp, pump_any))
        S[(0, 0)].flush_all()

        prev = (0, 0)
        seq = [(0, 1), (0, 2), (0, 3), (1, 3), (1, 0), (1, 1), (1, 2)]
        for qt, hp in seq:
            if (qt, hp) not in S:
                S[(qt, hp)] = SS(qt, hp, kT2s[hp], qproj(qt, hp, pump_any))
            if (qt, hp) == (1, 1):
                # create the final stream a step early so its exps can use
                # the ACT idle windows of this step, not just the last one
                S[(1, 2)] = SS(1, 2, kT2s[2], qproj(1, 2, pump_any))
            S[prev].flush_all()
            av_block(prev[0], prev[1], True)
            if prev == (0, 3):
                fc_queue.extend((0, qq) for qq in range(8))
            while len(fc_queue) > 4:
                emit_fc(True)
            prev = (qt, hp)

        S[prev].flush_all()
        fc_queue.extend((1, qq) for qq in range(8))
        av_block(prev[0], prev[1], False)
        while fc_queue:
            emit_fc()

    nc.compile()
    return nc


def _hilo(a):
    hi = a.astype(FP8_NP)
    lo = (a - hi.astype(np.float32)).astype(FP8_NP)
    return hi, lo


def _prep_inputs(inputs, W_qkv, W_fc, padding_mask, kj):
    KP = kj * 128
    x = np.asarray(inputs, np.float32)
    Wq = np.asarray(W_qkv, np.float32)
    Wf = np.asarray(W_fc, np.float32)
    mask = np.asarray(padding_mask)

    xT, xkT, keepc = {}, {}, {}
    for b in range(B):
        xb = x[b]
        xT[b] = _hilo(np.ascontiguousarray(xb.T))
        idx = np.nonzero(mask[b] == 0)[0]
        rows = np.zeros((KP, DIM), np.float32)
        rows[: len(idx)] = xb[idx]
        xkT[b] = _hilo(np.ascontiguousarray(rows.T))
        kv = np.zeros(KP, np.float32)
        kv[: len(idx)] = 1.0
        keepc[b] = np.ascontiguousarray(kv.reshape(kj, 128).T)

    in_maps = []
    for i in range(8):
        b, hs = i % 4, i // 4
        qrs = Wq[hs * 512 : (hs + 1) * 512]
        krs = Wq[DIM + hs * 512 : DIM + (hs + 1) * 512]
        vrs = Wq[2 * DIM + hs * 512 : 2 * DIM + (hs + 1) * 512]
        # column groups: all K slices first, then Q, then V - so the
        # startup-critical K weights are a small leading upload.
        wT = np.ascontiguousarray(
            np.concatenate([krs, qrs, vrs], axis=0).T
        ) * 16.0
        wh, wl = _hilo(wT)
        wfcT = np.concatenate(
            [
                np.ascontiguousarray(
                    Wf[:, hs * 512 + hp * 128 : hs * 512 + (hp + 1) * 128].T
                )
                for hp in range(NHP)
            ],
            axis=1,
        ) / 16.0
        in_maps.append(
            {
                "xTh": xT[b][0],
                "xTl": xT[b][1],
                "xkTh": xkT[b][0],
                "xkTl": xkT[b][1],
                "wqkvTh": wh,
                "wqkvTl": wl,
                "wfcT": wfcT.astype(BF16_NP),
                "keep": keepc[b],
            }
        )
    return in_maps


def kernel(inputs, W_qkv, W_fc, b_fc, padding_mask, trace=False, trace_kwargs=None):
    global LAST_RESULTS
    mask = np.asarray(padding_mask)
    kj = max(
        1, max(int(np.ceil((mask[b] == 0).sum() / 128)) for b in range(B))
    )
    if kj not in _CACHE:
        _CACHE[kj] = _build(kj)
    nc = _CACHE[kj]
    _CACHE["nc"] = nc  # last-used, for external profiling
    in_maps = _prep_inputs(inputs, W_qkv, W_fc, padding_mask, kj)
    kw = {}
    if trace:
        kw["trace"] = True
        if trace_kwargs:
            kw.update(trace_kwargs)
    res = run_bass_kernel_spmd(nc, in_maps, core_ids=list(range(8)), **kw)
    LAST_RESULTS = res
    out = np.empty((B, SEQ, DIM), np.float32)
    bfc = np.asarray(b_fc, np.float32)[None, :]
    for b in range(B):
        out[b] = (
            res.results[b]["outp"].astype(np.float32)
            + res.results[b + 4]["outp"].astype(np.float32)
            + bfc
        )
    return out


# revision 51
# speedup vs baseline: 1.0281x; 1.0007x over previous
"""Trainium2 Bass kernel for nn_Attention (B=4, SEQ=2048, DIM=1024, H=16).

Sharding v2: (batch x head-half) - core i handles batch i%4, heads
(i//4)*8 .. +8 (four head-pairs). Host sums the 2 partial FC outputs
per batch (+ b_fc). Versus the v1 all-batches-per-core tensor-parallel
split this keeps PE work identical but cuts per-core DMA ~4x (each
core loads only its batch) and removes the HWDGE/SP-queue contention.

Kernel structure per core:
- Host-side key compaction (padding-masked keys contribute exactly 0).
- QKV projections in fp8 hi/lo x DoubleRow (3 passes, 0.5 cyc/col).
- Scores (K=64) and AV (K=128) in bf16; exp on ACT engine.
- Softmax normalization: the AV PSUM rowsum row (augmented-V trick) is
  partition-broadcast on the idle GPSIMD engine directly from PSUM,
  reciprocal'd on DVE, and multiplied into xn - no PE broadcast matmul,
  no staging copies, no partition-shift DMA for the rowsum.
- FC: bf16, 4 accumulating K=128 chunk matmuls per output tile, chunk
  order chosen so the head-b partition-shift DMAs hide under the spine.
- Score exps are paced into AV/Qproj/FC gaps (~1.2us apart) so the ACT
  engine never back-pressures the in-order PE queue via the 2-deep
  score-PSUM pool.
"""

import sys

sys.path.insert(0, "/opt/trn_rl_repo")

from contextlib import ExitStack

import numpy as np
import ml_dtypes

import concourse.bass as bass
import concourse.tile as tile
from concourse import bacc, mybir
from concourse.bass_utils import run_bass_kernel_spmd

F32 = mybir.dt.float32
BF16 = mybir.dt.bfloat16
BF16_NP = ml_dtypes.bfloat16
FP8 = mybir.dt.float8e4
FP8_NP = ml_dtypes.float8_e4m3
DR = mybir.MatmulPerfMode.DoubleRow

B, SEQ, DIM, H, DH = 4, 2048, 1024, 16, 64
SCALE = DH ** -0.5  # 0.125
NHP = 4  # head-pairs per core (8 heads)

_CACHE = {}
LAST_RESULTS = None


def _build(kj):
    """kj: number of 128-wide key tiles after compaction (uniform, padded)."""
    KP = kj * 128

    nc = bacc.Bacc(
        "TRN2",
        target_bir_lowering=False,
        debug=False,
        enable_asserts=False,
        num_devices=8,
    )
    xTh = nc.dram_tensor("xTh", [DIM, SEQ], FP8, kind="ExternalInput").ap()
    xTl = nc.dram_tensor("xTl", [DIM, SEQ], FP8, kind="ExternalInput").ap()
    xkTh = nc.dram_tensor("xkTh", [DIM, KP], FP8, kind="ExternalInput").ap()
    xkTl = nc.dram_tensor("xkTl", [DIM, KP], FP8, kind="ExternalInput").ap()
    wqkvTh = nc.dram_tensor("wqkvTh", [DIM, 384 * NHP], FP8, kind="ExternalInput").ap()
    wqkvTl = nc.dram_tensor("wqkvTl", [DIM, 384 * NHP], FP8, kind="ExternalInput").ap()
    wfcT = nc.dram_tensor("wfcT", [128, NHP * DIM], BF16, kind="ExternalInput").ap()
    keep = nc.dram_tensor("keep", [128, kj], F32, kind="ExternalInput").ap()
    outp = nc.dram_tensor("outp", [SEQ, DIM], BF16, kind="ExternalOutput").ap()

    EXP = mybir.ActivationFunctionType.Exp

    with tile.TileContext(nc) as tc, ExitStack() as ctx:
        p_const = ctx.enter_context(tc.tile_pool(name="const", bufs=1))
        p_xq = ctx.enter_context(tc.tile_pool(name="xq", bufs=2))
        p_xk = ctx.enter_context(tc.tile_pool(name="xk", bufs=1))
        p_k = ctx.enter_context(tc.tile_pool(name="k", bufs=1))
        p_va = ctx.enter_context(tc.tile_pool(name="va", bufs=1))
        p_q = ctx.enter_context(tc.tile_pool(name="q", bufs=3))
        p_pt = ctx.enter_context(tc.tile_pool(name="pt", bufs=28))
        p_xn = ctx.enter_context(tc.tile_pool(name="xn", bufs=8))
        p_xnb = ctx.enter_context(tc.tile_pool(name="xnb", bufs=6))
        p_rb = ctx.enter_context(tc.tile_pool(name="rb", bufs=2))
        p_fo = ctx.enter_context(tc.tile_pool(name="fo", bufs=4))
        p_st = ctx.enter_context(tc.tile_pool(name="st", bufs=2, space="PSUM"))
        p_xa = ctx.enter_context(tc.tile_pool(name="xa", bufs=2, space="PSUM"))
        p_mm = ctx.enter_context(tc.tile_pool(name="mm", bufs=2, space="PSUM"))

        # ---- constant + input loads (SP queue) ----
        # The K-side tensors load in c-chunk halves, interleaved so the
        # pass-major startup K projection can start matmuls on (whA, xkhA)
        # ~4us in instead of waiting for all four full tensors.
        WQC = 384 * NHP
        wh_sb = p_const.tile([128, 8 * WQC], FP8, tag="wh")
        wl_sb = p_const.tile([128, 8 * WQC], FP8, tag="wl")
        xkh_sb = p_xk.tile([128, 8 * KP], FP8, tag="xkh")
        xkl_sb = p_xk.tile([128, 8 * KP], FP8, tag="xkl")

        def wcol_load(sb, src, a, b):
            nc.sync.dma_start(
                sb[:].rearrange("p (c n) -> p c n", c=8)[:, :, a:b],
                src[:, a:b].rearrange("(c p) n -> p c n", c=8),
            )

        def xk_half(sb, src, h):
            cs = slice(h * 4 * 128, (h + 1) * 4 * 128)
            nc.sync.dma_start(
                sb[:, h * 4 * KP : (h + 1) * 4 * KP].rearrange(
                    "p (c n) -> p c n", c=4
                ),
                src[cs, :].rearrange("(c p) n -> p c n", c=4),
            )

        wcol_load(wh_sb, wqkvTh, 0, 512)  # K weights (startup-critical)
        xk_half(xkh_sb, xkTh, 0)
        xk_half(xkh_sb, xkTh, 1)
        xk_half(xkl_sb, xkTl, 0)
        xk_half(xkl_sb, xkTl, 1)
        wcol_load(wl_sb, wqkvTl, 0, 512)
        wcol_load(wh_sb, wqkvTh, 512, 1024)  # Q weights
        wcol_load(wl_sb, wqkvTl, 512, 1024)
        keep_sb = p_const.tile([128, kj], F32, tag="keep")
        nc.sync.dma_start(keep_sb[:], keep[:])
        w3h = wh_sb[:].rearrange("p (c n) -> p c n", c=8)
        w3l = wl_sb[:].rearrange("p (c n) -> p c n", c=8)
        xk3h = xkh_sb[:].rearrange("p (c n) -> p c n", c=8)
        xk3l = xkl_sb[:].rearrange("p (c n) -> p c n", c=8)

        def load_xq(qt):
            t = {}
            cs = slice(qt * 1024, (qt + 1) * 1024)
            for tagv, src in (("xqh", xTh), ("xql", xTl)):
                xt = p_xq.tile([128, 8 * 1024], FP8, tag=tagv)
                nc.sync.dma_start(
                    xt[:].rearrange("p (c n) -> p c n", c=8),
                    src[:, cs].rearrange("(c p) n -> p c n", c=8),
                )
                t[tagv[-1]] = xt
            return t

        xq_t = {0: load_xq(0)}
        wcol_load(wh_sb, wqkvTh, 1024, 1536)  # V weights (needed latest)
        wcol_load(wl_sb, wqkvTl, 1024, 1536)
        xq_t[1] = load_xq(1)
        wfc_sb = p_const.tile([128, NHP * DIM], BF16, tag="wfc")
        nc.sync.dma_start(wfc_sb[:], wfcT[:])

        # ---- K projection: kT2[hp] = [128 (2 heads x 64 dh), KP] bf16 ----
        def kproj(hp):
            kT2 = p_k.tile([128, KP], BF16, tag=f"k{hp}")
            wo = hp * 128
            n0 = 0
            while n0 < KP:
                n1 = min(n0 + 512, KP)
                ps = p_mm.tile([128, 512], F32, tag="mm")
                first = True
                for wv, xv in ((w3h, xk3h), (w3h, xk3l), (w3l, xk3h)):
                    for cp in range(0, 8, 2):
                        nc.tensor.matmul(
                            ps[:, : n1 - n0],
                            wv[:, cp : cp + 2, wo : wo + 128],
                            xv[:, cp : cp + 2, n0:n1],
                            start=first,
                            stop=(wv is w3l and cp == 6),
                            perf_mode=DR,
                        )
                        first = False
                nc.scalar.copy(kT2[:, n0:n1], ps[:, : n1 - n0])
                n0 = n1
            return kT2

        # ---- V projection into keep-scaled augmented layout ----
        # va[hp] columns per key tile t: [v_a(64)*keep, keep, v_b(64)*keep, keep]
        def vproj(hp, pump):
            va = p_va.tile([128, kj * 130], BF16, tag=f"va{hp}")
            wo = 1024 + hp * 128
            for t in range(kj):
                pv = p_mm.tile([128, 128], F32, tag="mm")
                first = True
                for xv, wv in ((xk3h, w3h), (xk3l, w3h), (xk3h, w3l)):
                    for cp in range(0, 8, 2):
                        nc.tensor.matmul(
                            pv[:],
                            xv[:, cp : cp + 2, t * 128 : (t + 1) * 128],
                            wv[:, cp : cp + 2, wo : wo + 128],
                            start=first,
                            stop=(wv is w3l and cp == 6),
                            perf_mode=DR,
                        )
                        first = False
                kap = keep_sb[:, t : t + 1]
                o = t * 130
                nc.vector.tensor_scalar_mul(va[:, o : o + 64], pv[:, 0:64], kap)
                nc.vector.tensor_copy(va[:, o + 64 : o + 65], kap)
                nc.vector.tensor_scalar_mul(va[:, o + 65 : o + 129], pv[:, 64:128], kap)
                nc.vector.tensor_copy(va[:, o + 129 : o + 130], kap)
                if t % 2 == 1:
                    pump(1)
            return va

        # ---- Q projection: qS = [128 (2 heads x 64 dh), 1024 queries] ----
        def qproj(qt, hp, pump):
            qS = p_q.tile([128, 1024], BF16, tag="q")
            wo = 512 + hp * 128
            xq3h = xq_t[qt]["h"][:].rearrange("p (c n) -> p c n", c=8)
            xq3l = xq_t[qt]["l"][:].rearrange("p (c n) -> p c n", c=8)
            for n in range(2):
                ps = p_mm.tile([128, 512], F32, tag="mm")
                # xq-lo is the last load to arrive at startup, so the pass
                # that consumes it goes last.
                for pi, (wv, xv) in enumerate(
                    ((w3h, xq3h), (w3l, xq3h), (w3h, xq3l))
                ):
                    for cp in range(0, 8, 2):
                        nc.tensor.matmul(
                            ps[:],
                            wv[:, cp : cp + 2, wo : wo + 128],
                            xv[:, cp : cp + 2, n * 512 : (n + 1) * 512],
                            start=(pi == 0 and cp == 0),
                            stop=(pi == 2 and cp == 6),
                            perf_mode=DR,
                        )
                        if pi == 1 and cp == 6:
                            pump(1)
                nc.vector.tensor_copy(qS[:, n * 512 : (n + 1) * 512], ps[:])
                pump(1)
            return qS

        pts = {}
        xns = {}
        xnbs = {}
        kT2s = []
        vas = []
        fc_queue = []

        pending = []  # streams with unemitted score jobs, FIFO priority

        def pump_any(n=1):
            while n > 0 and pending:
                s = pending[0]
                if not s.jobs:
                    pending.pop(0)
                    continue
                s._emit(*s.jobs.pop(0))
                n -= 1

        def pending_jobs():
            return sum(len(s.jobs) for s in pending)

        class SS:
            """Pending score+exp tiles for one (qt, hp), paced into PE gaps."""

            def __init__(self, qt, hp, kT2, qS):
                self.qt, self.hp, self.kT2, self.qS = qt, hp, kT2, qS
                self.jobs = [(a, t) for a in range(2) for t in range(kj)]
                pending.append(self)

            def _emit(self, a, t):
                st = p_st.tile([128, 1024], F32, tag="st")
                for hh in range(2):
                    nc.tensor.matmul(
                        st[:, hh * 512 : (hh + 1) * 512],
                        self.kT2[a * 64 : (a + 1) * 64, t * 128 : (t + 1) * 128],
                        self.qS[a * 64 : (a + 1) * 64, hh * 512 : (hh + 1) * 512],
                        start=True,
                        stop=True,
                        skip_group_check=True,
                    )
                pt = p_pt.tile([128, 1024], BF16, tag="pt")
                nc.scalar.activation(pt[:], st[:], EXP, scale=SCALE / 256.0)
                pts[(self.qt, self.hp, a, t)] = pt

            def pump(self, n=1):
                while n > 0 and self.jobs:
                    self._emit(*self.jobs.pop(0))
                    n -= 1

            def flush_all(self):
                while self.jobs:
                    self._emit(*self.jobs.pop(0))

        def emit_fc(ss=None):
            if not fc_queue:
                return
            qt, qq = fc_queue.pop(0)
            last = qt == 1 and not fc_queue
            order = (0, 1, 2, 3) if qt == 0 else (3, 0, 1, 2)
            fo = p_fo.tile([128, DIM], BF16, tag="fo")
            r0 = qt * 1024 + qq * 128
            for ot in range(2):
                fp = p_mm.tile([128, 512], F32, tag="mm")
                for j, hp in enumerate(order):
                    nc.tensor.matmul(
                        fp[:],
                        xns[(qt, hp)][:, qq * 128 : (qq + 1) * 128],
                        wfc_sb[:, hp * DIM + ot * 512 : hp * DIM + ot * 512 + 512],
                        start=(j == 0),
                        stop=(j == 3),
                    )
                if qt == 1:
                    # qt1 entries drain at the tail: ACT is free of exps
                    # there, and this keeps DVE clear for the norm chain
                    nc.scalar.copy(fo[:, ot * 512 : (ot + 1) * 512], fp[:])
                else:
                    nc.vector.tensor_copy(fo[:, ot * 512 : (ot + 1) * 512], fp[:])
                if last:
                    # final tile: halves stored as staged via fast HWDGE
                    nc.sync.dma_start(
                        outp[r0 : r0 + 128, ot * 512 : (ot + 1) * 512],
                        fo[:, ot * 512 : (ot + 1) * 512],
                    )
                if ss:
                    pump_any(1)
            if not last:
                nc.gpsimd.dma_start(outp[r0 : r0 + 128, :], fo[:])

        def av_block(qt, hp, ss_next):
            xn = p_xn.tile([128, 1024], BF16, tag="xn", name=f"xn{qt}{hp}")
            xns[(qt, hp)] = xn
            va = vas[hp]
            for a in range(2):
                for qh in range(2):
                    xa = p_xa.tile([65, 512], F32, tag="xa")
                    for t in range(kj):
                        o = t * 130 + a * 65
                        nc.tensor.matmul(
                            xa[:],
                            va[:, o : o + 65],
                            pts[(qt, hp, a, t)][:, qh * 512 : (qh + 1) * 512],
                            start=(t == 0),
                            stop=(t == kj - 1),
                            skip_group_check=True,
                        )
                        if t % 4 == 3 and ss_next and pending_jobs() > 10:
                            pump_any(1)
                    # normalize: stage the PSUM rowsum row to SBUF, take the
                    # reciprocal on DVE, partition-broadcast on GPSIMD,
                    # multiply into xn.
                    rs = p_rb.tile([1, 512], F32, tag="rs")
                    nc.vector.tensor_copy(rs[0:1, :], xa[64:65, :])
                    rr = p_rb.tile([1, 512], F32, tag="rr")
                    nc.vector.reciprocal_approx_fast(rr[0:1, :], rs[0:1, :])
                    ri = p_rb.tile([64, 512], F32, tag="ri")
                    nc.gpsimd.partition_broadcast(ri[:], rr[0:1, :])
                    if a == 0:
                        nc.vector.tensor_mul(
                            xn[0:64, qh * 512 : (qh + 1) * 512], xa[0:64, :], ri[:]
                        )
                    else:
                        xnb = p_xnb.tile([64, 512], BF16, tag="xnb")
                        nc.vector.tensor_mul(xnb[:], xa[0:64, :], ri[:])
                        # shift DMA on SP (stores go via GPSIMD SWDGE so
                        # they cannot delay these latency-critical shifts)
                        nc.sync.dma_start(
                            xn[64:128, qh * 512 : (qh + 1) * 512], xnb[:]
                        )
                        emit_fc(ss_next)
                        emit_fc(ss_next)
                    if ss_next:
                        pump_any(3)

        # hp0+hp1 K projections pass-major: the hh-pass matmuls for all
        # chunks of both head-pairs run as soon as (wh, xkh) land, the
        # hl-pass when xkl lands, the lh-pass when wl lands - instead of
        # the whole pipeline stalling on the last of the four loads.
        def kproj01():
            res, ps2, pc2 = [], {}, {}
            nch = [
                (i * 512, min((i + 1) * 512, KP))
                for i in range((KP + 511) // 512)
            ]
            for hp in (0, 1):
                res.append(
                    p_k.tile([128, KP], BF16, tag=f"k{hp}", name=f"kT2_{hp}")
                )
                ps2[hp] = p_st.tile([128, 1024], F32, tag="st", name=f"kps{hp}")
                if KP > 1024:
                    pc2[hp] = p_mm.tile([128, 512], F32, tag="mm", name=f"kpc{hp}")
            for pi, (wv, xv) in enumerate(
                ((w3h, xk3h), (w3h, xk3l), (w3l, xk3h))
            ):
                for cph in (0, 4):
                    for hp in (0, 1):
                        wo = hp * 128
                        for ci, (n0, n1) in enumerate(nch):
                            out = (
                                ps2[hp][:, n0:n1]
                                if ci < 2
                                else pc2[hp][:, : n1 - n0]
                            )
                            for cp in (cph, cph + 2):
                                nc.tensor.matmul(
                                    out,
                                    wv[:, cp : cp + 2, wo : wo + 128],
                                    xv[:, cp : cp + 2, n0:n1],
                                    start=(pi == 0 and cp == 0),
                                    stop=(pi == 2 and cp == 6),
                                    perf_mode=DR,
                                    skip_group_check=True,
                                )
            for hp in (0, 1):
                e = min(KP, 1024)
                # ACT is idle during startup - keep DVE free for qS copies
                nc.scalar.copy(res[hp][:, 0:e], ps2[hp][:, 0:e])
                if KP > 1024:
                    nc.scalar.copy(res[hp][:, 1024:KP], pc2[hp][:, : KP - 1024])
            return res

        # ================= spine =================
        if KP <= 1536:
            kT2s.extend(kproj01())
        else:
            kT2s.append(kproj(0))
            kT2s.append(kproj(1))
        kT2s.append(kproj(2))
        kT2s.append(kproj(3))

        qS0 = qproj(0, 0, lambda n: None)
        S = {(0, 0): SS(0, 0, kT2s[0], qS0)}

        for hp in range(NHP):
            vas.append(vproj(hp, pump_any))
        S[(0, 0)].flush_all()

        prev = (0, 0)
        seq = [(0, 1), (0, 2), (0, 3), (1, 3), (1, 0), (1, 1), (1, 2)]
        for qt, hp in seq:
            if (qt, hp) not in S:
                S[(qt, hp)] = SS(qt, hp, kT2s[hp], qproj(qt, hp, pump_any))
            if (qt, hp) == (1, 1):
                # create the final stream a step early so its exps can use
                # the ACT idle windows of this step, not just the last one
                S[(1, 2)] = SS(1, 2, kT2s[2], qproj(1, 2, pump_any))
            S[prev].flush_all()
            av_block(prev[0], prev[1], True)
            if prev == (0, 3):
                fc_queue.extend((0, qq) for qq in range(8))
            while len(fc_queue) > 4:
                emit_fc(True)
            prev = (qt, hp)

        S[prev].flush_all()
        fc_queue.extend((1, qq) for qq in range(8))
        av_block(prev[0], prev[1], False)
        while fc_queue:
            emit_fc()

    nc.compile()
    return nc


def _hilo(a):
    hi = a.astype(FP8_NP)
    lo = (a - hi.astype(np.float32)).astype(FP8_NP)
    return hi, lo


def _prep_inputs(inputs, W_qkv, W_fc, padding_mask, kj):
    KP = kj * 128
    x = np.asarray(inputs, np.float32)
    Wq = np.asarray(W_qkv, np.float32)
    Wf = np.asarray(W_fc, np.float32)
    mask = np.asarray(padding_mask)

    xT, xkT, keepc = {}, {}, {}
    for b in range(B):
        xb = x[b]
        xT[b] = _hilo(np.ascontiguousarray(xb.T))
        idx = np.nonzero(mask[b] == 0)[0]
        rows = np.zeros((KP, DIM), np.float32)
        rows[: len(idx)] = xb[idx]
        xkT[b] = _hilo(np.ascontiguousarray(rows.T))
        kv = np.zeros(KP, np.float32)
        kv[: len(idx)] = 1.0
        keepc[b] = np.ascontiguousarray(kv.reshape(kj, 128).T)

    in_maps = []
    for i in range(8):
        b, hs = i % 4, i // 4
        qrs = Wq[hs * 512 : (hs + 1) * 512]
        krs = Wq[DIM + hs * 512 : DIM + (hs + 1) * 512]
        vrs = Wq[2 * DIM + hs * 512 : 2 * DIM + (hs + 1) * 512]
        # column groups: all K slices first, then Q, then V - so the
        # startup-critical K weights are a small leading upload.
        wT = np.ascontiguousarray(
            np.concatenate([krs, qrs, vrs], axis=0).T
        ) * 16.0
        wh, wl = _hilo(wT)
        wfcT = np.concatenate(
            [
                np.ascontiguousarray(
                    Wf[:, hs * 512 + hp * 128 : hs * 512 + (hp + 1) * 128].T
                )
                for hp in range(NHP)
            ],
            axis=1,
        ) / 16.0
        in_maps.append(
            {
                "xTh": xT[b][0],
                "xTl": xT[b][1],
                "xkTh": xkT[b][0],
                "xkTl": xkT[b][1],
                "wqkvTh": wh,
                "wqkvTl": wl,
                "wfcT": wfcT.astype(BF16_NP),
                "keep": keepc[b],
            }
        )
    return in_maps


def kernel(inputs, W_qkv, W_fc, b_fc, padding_mask, trace=False, trace_kwargs=None):
    global LAST_RESULTS
    mask = np.asarray(padding_mask)
    kj = max(
        1, max(int(np.ceil((mask[b] == 0).sum() / 128)) for b in range(B))
    )
    if kj not in _CACHE:
        _CACHE[kj] = _build(kj)
    nc = _CACHE[kj]
    _CACHE["nc"] = nc  # last-used, for external profiling
    in_maps = _prep_inputs(inputs, W_qkv, W_fc, padding_mask, kj)
    kw = {}
    if trace:
        kw["trace"] = True
        if trace_kwargs:
            kw.update(trace_kwargs)
    res = run_bass_kernel_spmd(nc, in_maps, core_ids=list(range(8)), **kw)
    LAST_RESULTS = res
    out = np.empty((B, SEQ, DIM), np.float32)
    bfc = np.asarray(b_fc, np.float32)[None, :]
    for b in range(B):
        out[b] = (
            res.results[b]["outp"].astype(np.float32)
            + res.results[b + 4]["outp"].astype(np.float32)
            + bfc
        )
    return out


# revision 58
# speedup vs baseline: 1.0283x; 1.0002x over previous
"""Trainium2 Bass kernel for nn_Attention (B=4, SEQ=2048, DIM=1024, H=16).

Sharding: (batch x head-half) - core i handles batch i%4 and heads
(i//4)*8..+8 (four head-pairs, all 2048 queries). The host sums the two
partial FC outputs per batch (+ b_fc). Versus an all-batches-per-core
head-parallel split this keeps per-core PE work identical but cuts
per-core DMA ~4x (each core only loads its own batch) and removes the
HWDGE/SP-queue contention that dominated the v1 kernel.

Per-core structure:
- Host-side key compaction: padding-masked keys contribute exactly 0
  (exp(-1e7) == 0 in fp32), so K/V projection, scores, exp and AV run
  only over kept keys, padded to a uniform 128-multiple across cores.
- QKV projections in fp8 hi/lo x DoubleRow (3 passes at 0.5 cyc/col
  beats one bf16 pass at 1.0); scores (K=64) and AV (K=128) in bf16 -
  measured: any single-sided fp8 in scores/AV exceeds the 2e-2 gate.
- Softmax normalization: the AV PSUM rowsum row (augmented-V trick) is
  staged to SBUF, reciprocal'd on DVE, partition-broadcast on the idle
  GPSIMD engine, and multiplied into xn. No PE broadcast matmul.
- Startup: weights upload in [K|Q|V] column groups and xk in halves so
  the pass-major K projection starts matmuls as soon as (wK_hi, xk_hi)
  land, continuing as each later tensor arrives.
- Score exps are paced into AV/Qproj/FC gaps (~1.1us apart) so the ACT
  engine never back-pressures the in-order PE queue through the 2-deep
  score-PSUM pool; the final stream is created a step early.
- DMA queues: loads + latency-critical xn partition shifts on SP,
  output stores on the GPSIMD SWDGE path; qt1 FC staging copies run on
  ACT (drained of exps by then) to keep DVE clear for the norm chain.
"""

import sys

sys.path.insert(0, "/opt/trn_rl_repo")

from contextlib import ExitStack

import numpy as np
import ml_dtypes

import concourse.bass as bass
import concourse.tile as tile
from concourse import bacc, mybir
from concourse.bass_utils import run_bass_kernel_spmd

F32 = mybir.dt.float32
BF16 = mybir.dt.bfloat16
BF16_NP = ml_dtypes.bfloat16
FP8 = mybir.dt.float8e4
FP8_NP = ml_dtypes.float8_e4m3
DR = mybir.MatmulPerfMode.DoubleRow

B, SEQ, DIM, H, DH = 4, 2048, 1024, 16, 64
SCALE = DH ** -0.5  # 0.125
NHP = 4  # head-pairs per core (8 heads)

_CACHE = {}
LAST_RESULTS = None


def _build(kj):
    """kj: number of 128-wide key tiles after compaction (uniform, padded)."""
    KP = kj * 128

    nc = bacc.Bacc(
        "TRN2",
        target_bir_lowering=False,
        debug=False,
        enable_asserts=False,
        num_devices=8,
    )
    xTh = nc.dram_tensor("xTh", [DIM, SEQ], FP8, kind="ExternalInput").ap()
    xTl = nc.dram_tensor("xTl", [DIM, SEQ], FP8, kind="ExternalInput").ap()
    xkTh = nc.dram_tensor("xkTh", [DIM, KP], FP8, kind="ExternalInput").ap()
    xkTl = nc.dram_tensor("xkTl", [DIM, KP], FP8, kind="ExternalInput").ap()
    wqkvTh = nc.dram_tensor("wqkvTh", [DIM, 384 * NHP], FP8, kind="ExternalInput").ap()
    wqkvTl = nc.dram_tensor("wqkvTl", [DIM, 384 * NHP], FP8, kind="ExternalInput").ap()
    wfcT = nc.dram_tensor("wfcT", [128, NHP * DIM], BF16, kind="ExternalInput").ap()
    keep = nc.dram_tensor("keep", [128, kj], F32, kind="ExternalInput").ap()
    outp = nc.dram_tensor("outp", [SEQ, DIM], BF16, kind="ExternalOutput").ap()

    EXP = mybir.ActivationFunctionType.Exp

    with tile.TileContext(nc) as tc, ExitStack() as ctx:
        p_const = ctx.enter_context(tc.tile_pool(name="const", bufs=1))
        p_xq = ctx.enter_context(tc.tile_pool(name="xq", bufs=2))
        p_xk = ctx.enter_context(tc.tile_pool(name="xk", bufs=1))
        p_k = ctx.enter_context(tc.tile_pool(name="k", bufs=1))
        p_va = ctx.enter_context(tc.tile_pool(name="va", bufs=1))
        p_q = ctx.enter_context(tc.tile_pool(name="q", bufs=3))
        p_pt = ctx.enter_context(tc.tile_pool(name="pt", bufs=28))
        p_xn = ctx.enter_context(tc.tile_pool(name="xn", bufs=8))
        p_xnb = ctx.enter_context(tc.tile_pool(name="xnb", bufs=6))
        p_rb = ctx.enter_context(tc.tile_pool(name="rb", bufs=2))
        p_fo = ctx.enter_context(tc.tile_pool(name="fo", bufs=4))
        p_st = ctx.enter_context(tc.tile_pool(name="st", bufs=2, space="PSUM"))
        p_xa = ctx.enter_context(tc.tile_pool(name="xa", bufs=2, space="PSUM"))
        p_mm = ctx.enter_context(tc.tile_pool(name="mm", bufs=2, space="PSUM"))

        # ---- constant + input loads (SP queue) ----
        # The K-side tensors load in c-chunk halves, interleaved so the
        # pass-major startup K projection can start matmuls on (whA, xkhA)
        # ~4us in instead of waiting for all four full tensors.
        WQC = 384 * NHP
        wh_sb = p_const.tile([128, 8 * WQC], FP8, tag="wh")
        wl_sb = p_const.tile([128, 8 * WQC], FP8, tag="wl")
        xkh_sb = p_xk.tile([128, 8 * KP], FP8, tag="xkh")
        xkl_sb = p_xk.tile([128, 8 * KP], FP8, tag="xkl")

        def wcol_load(sb, src, a, b):
            nc.sync.dma_start(
                sb[:].rearrange("p (c n) -> p c n", c=8)[:, :, a:b],
                src[:, a:b].rearrange("(c p) n -> p c n", c=8),
            )

        def xk_half(sb, src, h):
            cs = slice(h * 4 * 128, (h + 1) * 4 * 128)
            nc.sync.dma_start(
                sb[:, h * 4 * KP : (h + 1) * 4 * KP].rearrange(
                    "p (c n) -> p c n", c=4
                ),
                src[cs, :].rearrange("(c p) n -> p c n", c=4),
            )

        wcol_load(wh_sb, wqkvTh, 0, 512)  # K weights (startup-critical)
        xk_half(xkh_sb, xkTh, 0)
        xk_half(xkh_sb, xkTh, 1)
        xk_half(xkl_sb, xkTl, 0)
        xk_half(xkl_sb, xkTl, 1)
        wcol_load(wl_sb, wqkvTl, 0, 512)
        wcol_load(wh_sb, wqkvTh, 512, 1024)  # Q weights
        wcol_load(wl_sb, wqkvTl, 512, 1024)
        keep_sb = p_const.tile([128, kj], F32, tag="keep")
        nc.sync.dma_start(keep_sb[:], keep[:])
        w3h = wh_sb[:].rearrange("p (c n) -> p c n", c=8)
        w3l = wl_sb[:].rearrange("p (c n) -> p c n", c=8)
        xk3h = xkh_sb[:].rearrange("p (c n) -> p c n", c=8)
        xk3l = xkl_sb[:].rearrange("p (c n) -> p c n", c=8)

        def load_xq(qt):
            t = {}
            cs = slice(qt * 1024, (qt + 1) * 1024)
            for tagv, src in (("xqh", xTh), ("xql", xTl)):
                xt = p_xq.tile([128, 8 * 1024], FP8, tag=tagv)
                nc.sync.dma_start(
                    xt[:].rearrange("p (c n) -> p c n", c=8),
                    src[:, cs].rearrange("(c p) n -> p c n", c=8),
                )
                t[tagv[-1]] = xt
            return t

        wcol_load(wh_sb, wqkvTh, 1024, 1536)  # V weights
        xq_t = {0: load_xq(0)}
        wcol_load(wl_sb, wqkvTl, 1024, 1536)
        xq_t[1] = load_xq(1)
        wfc_sb = p_const.tile([128, NHP * DIM], BF16, tag="wfc")
        nc.sync.dma_start(wfc_sb[:], wfcT[:])

        # ---- K projection: kT2[hp] = [128 (2 heads x 64 dh), KP] bf16 ----
        def kproj(hp):
            kT2 = p_k.tile([128, KP], BF16, tag=f"k{hp}")
            wo = hp * 128
            n0 = 0
            while n0 < KP:
                n1 = min(n0 + 512, KP)
                ps = p_mm.tile([128, 512], F32, tag="mm")
                first = True
                for wv, xv in ((w3h, xk3h), (w3h, xk3l), (w3l, xk3h)):
                    for cp in range(0, 8, 2):
                        nc.tensor.matmul(
                            ps[:, : n1 - n0],
                            wv[:, cp : cp + 2, wo : wo + 128],
                            xv[:, cp : cp + 2, n0:n1],
                            start=first,
                            stop=(wv is w3l and cp == 6),
                            perf_mode=DR,
                        )
                        first = False
                nc.scalar.copy(kT2[:, n0:n1], ps[:, : n1 - n0])
                n0 = n1
            return kT2

        # ---- V projection into keep-scaled augmented layout ----
        # va[hp] columns per key tile t: [v_a(64)*keep, keep, v_b(64)*keep, keep]
        def vproj(hp, pump):
            va = p_va.tile([128, kj * 130], BF16, tag=f"va{hp}")
            wo = 1024 + hp * 128
            for t in range(kj):
                pv = p_mm.tile([128, 128], F32, tag="mm")
                first = True
                for xv, wv in ((xk3h, w3h), (xk3l, w3h), (xk3h, w3l)):
                    for cp in range(0, 8, 2):
                        nc.tensor.matmul(
                            pv[:],
                            xv[:, cp : cp + 2, t * 128 : (t + 1) * 128],
                            wv[:, cp : cp + 2, wo : wo + 128],
                            start=first,
                            stop=(wv is w3l and cp == 6),
                            perf_mode=DR,
                        )
                        first = False
                kap = keep_sb[:, t : t + 1]
                o = t * 130
                nc.vector.tensor_scalar_mul(va[:, o : o + 64], pv[:, 0:64], kap)
                nc.vector.tensor_copy(va[:, o + 64 : o + 65], kap)
                nc.vector.tensor_scalar_mul(va[:, o + 65 : o + 129], pv[:, 64:128], kap)
                nc.vector.tensor_copy(va[:, o + 129 : o + 130], kap)
                if t % 2 == 1:
                    pump(1)
            return va

        # ---- Q projection: qS = [128 (2 heads x 64 dh), 1024 queries] ----
        def qproj(qt, hp, pump):
            qS = p_q.tile([128, 1024], BF16, tag="q")
            wo = 512 + hp * 128
            xq3h = xq_t[qt]["h"][:].rearrange("p (c n) -> p c n", c=8)
            xq3l = xq_t[qt]["l"][:].rearrange("p (c n) -> p c n", c=8)
            for n in range(2):
                ps = p_mm.tile([128, 512], F32, tag="mm")
                # xq-lo is the last load to arrive at startup, so the pass
                # that consumes it goes last.
                for pi, (wv, xv) in enumerate(
                    ((w3h, xq3h), (w3l, xq3h), (w3h, xq3l))
                ):
                    for cp in range(0, 8, 2):
                        nc.tensor.matmul(
                            ps[:],
                            wv[:, cp : cp + 2, wo : wo + 128],
                            xv[:, cp : cp + 2, n * 512 : (n + 1) * 512],
                            start=(pi == 0 and cp == 0),
                            stop=(pi == 2 and cp == 6),
                            perf_mode=DR,
                        )
                        if pi == 1 and cp == 6:
                            pump(1)
                nc.vector.tensor_copy(qS[:, n * 512 : (n + 1) * 512], ps[:])
                pump(1)
            return qS

        pts = {}
        xns = {}
        kT2s = []
        vas = []
        fc_queue = []

        pending = []  # streams with unemitted score jobs, FIFO priority

        def pump_any(n=1):
            while n > 0 and pending:
                s = pending[0]
                if not s.jobs:
                    pending.pop(0)
                    continue
                s._emit(*s.jobs.pop(0))
                n -= 1

        def pending_jobs():
            return sum(len(s.jobs) for s in pending)

        class SS:
            """Pending score+exp tiles for one (qt, hp), paced into PE gaps."""

            def __init__(self, qt, hp, kT2, qS):
                self.qt, self.hp, self.kT2, self.qS = qt, hp, kT2, qS
                self.jobs = [(a, t) for a in range(2) for t in range(kj)]
                pending.append(self)

            def _emit(self, a, t):
                st = p_st.tile([128, 1024], F32, tag="st")
                for hh in range(2):
                    nc.tensor.matmul(
                        st[:, hh * 512 : (hh + 1) * 512],
                        self.kT2[a * 64 : (a + 1) * 64, t * 128 : (t + 1) * 128],
                        self.qS[a * 64 : (a + 1) * 64, hh * 512 : (hh + 1) * 512],
                        start=True,
                        stop=True,
                        skip_group_check=True,
                    )
                pt = p_pt.tile([128, 1024], BF16, tag="pt")
                nc.scalar.activation(pt[:], st[:], EXP, scale=SCALE / 256.0)
                pts[(self.qt, self.hp, a, t)] = pt

            def pump(self, n=1):
                while n > 0 and self.jobs:
                    self._emit(*self.jobs.pop(0))
                    n -= 1

            def flush_all(self):
                while self.jobs:
                    self._emit(*self.jobs.pop(0))

        def emit_fc(ss=None):
            if not fc_queue:
                return
            qt, qq = fc_queue.pop(0)
            last = qt == 1 and not fc_queue
            order = (0, 1, 2, 3) if qt == 0 else (3, 0, 1, 2)
            fo = p_fo.tile([128, DIM], BF16, tag="fo")
            r0 = qt * 1024 + qq * 128
            for ot in range(2):
                fp = p_mm.tile([128, 512], F32, tag="mm")
                for j, hp in enumerate(order):
                    nc.tensor.matmul(
                        fp[:],
                        xns[(qt, hp)][:, qq * 128 : (qq + 1) * 128],
                        wfc_sb[:, hp * DIM + ot * 512 : hp * DIM + ot * 512 + 512],
                        start=(j == 0),
                        stop=(j == 3),
                    )
                if qt == 1:
                    # qt1 entries drain at the tail: ACT is free of exps
                    # there, and this keeps DVE clear for the norm chain
                    nc.scalar.copy(fo[:, ot * 512 : (ot + 1) * 512], fp[:])
                else:
                    nc.vector.tensor_copy(fo[:, ot * 512 : (ot + 1) * 512], fp[:])
                if last:
                    # final tile: halves stored as staged via fast HWDGE
                    nc.sync.dma_start(
                        outp[r0 : r0 + 128, ot * 512 : (ot + 1) * 512],
                        fo[:, ot * 512 : (ot + 1) * 512],
                    )
                if ss:
                    pump_any(1)
            if not last:
                nc.gpsimd.dma_start(outp[r0 : r0 + 128, :], fo[:])

        def av_block(qt, hp, ss_next):
            xn = p_xn.tile([128, 1024], BF16, tag="xn", name=f"xn{qt}{hp}")
            xns[(qt, hp)] = xn
            va = vas[hp]
            for a in range(2):
                for qh in range(2):
                    xa = p_xa.tile([65, 512], F32, tag="xa")
                    for t in range(kj):
                        o = t * 130 + a * 65
                        nc.tensor.matmul(
                            xa[:],
                            va[:, o : o + 65],
                            pts[(qt, hp, a, t)][:, qh * 512 : (qh + 1) * 512],
                            start=(t == 0),
                            stop=(t == kj - 1),
                            skip_group_check=True,
                        )
                        if t % 2 == 1 and ss_next and pending_jobs() > 6:
                            pump_any(1)
                    # normalize: stage the PSUM rowsum row to SBUF, take the
                    # reciprocal on DVE, partition-broadcast on GPSIMD,
                    # multiply into xn.
                    rs = p_rb.tile([1, 512], F32, tag="rs")
                    nc.vector.tensor_copy(rs[0:1, :], xa[64:65, :])
                    rr = p_rb.tile([1, 512], F32, tag="rr")
                    nc.vector.reciprocal_approx_fast(rr[0:1, :], rs[0:1, :])
                    ri = p_rb.tile([64, 512], F32, tag="ri")
                    nc.gpsimd.partition_broadcast(ri[:], rr[0:1, :])
                    if a == 0:
                        nc.vector.tensor_mul(
                            xn[0:64, qh * 512 : (qh + 1) * 512], xa[0:64, :], ri[:]
                        )
                    else:
                        xnb = p_xnb.tile([64, 512], BF16, tag="xnb")
                        nc.vector.tensor_mul(xnb[:], xa[0:64, :], ri[:])
                        # shift DMA on SP (stores go via GPSIMD SWDGE so
                        # they cannot delay these latency-critical shifts)
                        nc.sync.dma_start(
                            xn[64:128, qh * 512 : (qh + 1) * 512], xnb[:]
                        )
                        emit_fc(ss_next)
                        emit_fc(ss_next)
                    if ss_next:
                        pump_any(4)

        # hp0+hp1 K projections pass-major: the hh-pass matmuls for all
        # chunks of both head-pairs run as soon as (wh, xkh) land, the
        # hl-pass when xkl lands, the lh-pass when wl lands - instead of
        # the whole pipeline stalling on the last of the four loads.
        def kproj01():
            res, ps2, pc2 = [], {}, {}
            nch = [
                (i * 512, min((i + 1) * 512, KP))
                for i in range((KP + 511) // 512)
            ]
            for hp in (0, 1):
                res.append(
                    p_k.tile([128, KP], BF16, tag=f"k{hp}", name=f"kT2_{hp}")
                )
                ps2[hp] = p_st.tile([128, 1024], F32, tag="st", name=f"kps{hp}")
                if KP > 1024:
                    pc2[hp] = p_mm.tile([128, 512], F32, tag="mm", name=f"kpc{hp}")
            for pi, (wv, xv) in enumerate(
                ((w3h, xk3h), (w3h, xk3l), (w3l, xk3h))
            ):
                for cph in (0, 4):
                    for hp in (0, 1):
                        wo = hp * 128
                        for ci, (n0, n1) in enumerate(nch):
                            out = (
                                ps2[hp][:, n0:n1]
                                if ci < 2
                                else pc2[hp][:, : n1 - n0]
                            )
                            for cp in (cph, cph + 2):
                                nc.tensor.matmul(
                                    out,
                                    wv[:, cp : cp + 2, wo : wo + 128],
                                    xv[:, cp : cp + 2, n0:n1],
                                    start=(pi == 0 and cp == 0),
                                    stop=(pi == 2 and cp == 6),
                                    perf_mode=DR,
                                    skip_group_check=True,
                                )
            for hp in (0, 1):
                e = min(KP, 1024)
                # ACT is idle during startup - keep DVE free for qS copies
                nc.scalar.copy(res[hp][:, 0:e], ps2[hp][:, 0:e])
                if KP > 1024:
                    nc.scalar.copy(res[hp][:, 1024:KP], pc2[hp][:, : KP - 1024])
            return res

        # ================= spine =================
        if KP <= 1536:
            kT2s.extend(kproj01())
        else:
            kT2s.append(kproj(0))
            kT2s.append(kproj(1))
        kT2s.append(kproj(2))
        kT2s.append(kproj(3))

        qS0 = qproj(0, 0, lambda n: None)
        S = {(0, 0): SS(0, 0, kT2s[0], qS0)}

        for hp in range(NHP):
            vas.append(vproj(hp, pump_any))
        S[(0, 0)].flush_all()

        prev = (0, 0)
        seq = [(0, 1), (0, 2), (0, 3), (1, 3), (1, 0), (1, 1), (1, 2)]
        for qt, hp in seq:
            if (qt, hp) not in S:
                S[(qt, hp)] = SS(qt, hp, kT2s[hp], qproj(qt, hp, pump_any))
            if (qt, hp) == (1, 1):
                # create the final stream a step early so its exps can use
                # the ACT idle windows of this step, not just the last one
                S[(1, 2)] = SS(1, 2, kT2s[2], qproj(1, 2, pump_any))
            S[prev].flush_all()
            av_block(prev[0], prev[1], True)
            if prev == (0, 3):
                fc_queue.extend((0, qq) for qq in range(8))
            while len(fc_queue) > 4:
                emit_fc(True)
            prev = (qt, hp)

        S[prev].flush_all()
        fc_queue.extend((1, qq) for qq in range(8))
        av_block(prev[0], prev[1], False)
        while fc_queue:
            emit_fc()

    nc.compile()
    return nc


def _hilo(a):
    hi = a.astype(FP8_NP)
    lo = (a - hi.astype(np.float32)).astype(FP8_NP)
    return hi, lo


def _prep_inputs(inputs, W_qkv, W_fc, padding_mask, kj):
    KP = kj * 128
    x = np.asarray(inputs, np.float32)
    Wq = np.asarray(W_qkv, np.float32)
    Wf = np.asarray(W_fc, np.float32)
    mask = np.asarray(padding_mask)

    xT, xkT, keepc = {}, {}, {}
    for b in range(B):
        xb = x[b]
        xT[b] = _hilo(np.ascontiguousarray(xb.T))
        idx = np.nonzero(mask[b] == 0)[0]
        rows = np.zeros((KP, DIM), np.float32)
        rows[: len(idx)] = xb[idx]
        xkT[b] = _hilo(np.ascontiguousarray(rows.T))
        kv = np.zeros(KP, np.float32)
        kv[: len(idx)] = 1.0
        keepc[b] = np.ascontiguousarray(kv.reshape(kj, 128).T)

    in_maps = []
    for i in range(8):
        b, hs = i % 4, i // 4
        qrs = Wq[hs * 512 : (hs + 1) * 512]
        krs = Wq[DIM + hs * 512 : DIM + (hs + 1) * 512]
        vrs = Wq[2 * DIM + hs * 512 : 2 * DIM + (hs + 1) * 512]
        # column groups: all K slices first, then Q, then V - so the
        # startup-critical K weights are a small leading upload.
        wT = np.ascontiguousarray(
            np.concatenate([krs, qrs, vrs], axis=0).T
        ) * 16.0
        wh, wl = _hilo(wT)
        wfcT = np.concatenate(
            [
                np.ascontiguousarray(
                    Wf[:, hs * 512 + hp * 128 : hs * 512 + (hp + 1) * 128].T
                )
                for hp in range(NHP)
            ],
            axis=1,
        ) / 16.0
        in_maps.append(
            {
                "xTh": xT[b][0],
                "xTl": xT[b][1],
                "xkTh": xkT[b][0],
                "xkTl": xkT[b][1],
                "wqkvTh": wh,
                "wqkvTl": wl,
                "wfcT": wfcT.astype(BF16_NP),
                "keep": keepc[b],
            }
        )
    return in_maps


def kernel(inputs, W_qkv, W_fc, b_fc, padding_mask, trace=False, trace_kwargs=None):
    global LAST_RESULTS
    mask = np.asarray(padding_mask)
    kj = max(
        1, max(int(np.ceil((mask[b] == 0).sum() / 128)) for b in range(B))
    )
    if kj not in _CACHE:
        _CACHE[kj] = _build(kj)
    nc = _CACHE[kj]
    _CACHE["nc"] = nc  # last-used, for external profiling
    in_maps = _prep_inputs(inputs, W_qkv, W_fc, padding_mask, kj)
    kw = {}
    if trace:
        kw["trace"] = True
        if trace_kwargs:
            kw.update(trace_kwargs)
    res = run_bass_kernel_spmd(nc, in_maps, core_ids=list(range(8)), **kw)
    LAST_RESULTS = res
    out = np.empty((B, SEQ, DIM), np.float32)
    bfc = np.asarray(b_fc, np.float32)[None, :]
    for b in range(B):
        out[b] = (
            res.results[b]["outp"].astype(np.float32)
            + res.results[b + 4]["outp"].astype(np.float32)
            + bfc
        )
    return out
